# revision 27
# baseline (speedup 1.0000x reference)
"""Multi-head causal attention (B=4, T=2048, D=1024, H=16) on 8 TRN2 NeuronCores.

Sharding: 8 cores = 4 batches x 2 head-halves. Core c handles batch c//2 and
heads [ (c%2)*8, (c%2)*8+8 ).  Each core computes its half of the attention
output and its partial output projection; the host sums the two partial
projections per batch.

Per-core device kernel (matmul inputs bf16, fp32 PSUM accumulation):
  phase A (per head-pair): Q^T, K^T [128ch x 2048t] slices; once: V (natural
        [2048t x 512ch] layout, stored with a ones-column per head so the
        PV matmul also produces the softmax denominator l)
  phase B (per head-pair, per 512-query block): causal flash attention:
        S^T tiles [128k x 1024(2 heads)] -> one exp -> staircase mask mul on
        diagonal tiles -> O^T accumulation in PSUM (65 rows: 64 out + l)
        -> fast PSUM release via ACT copy; 1/l via approx reciprocal +
        gpsimd partition-broadcast, off the critical path
  phase C: partial output projection out[t, 1024] = attn_half @ W_o_half

Emission order interleaves phase-A work of pair p+1 after phase B of pair p so
the TensorE always has fill work (keeps the HAM clock-gate warm).
"""

import numpy as np
import ml_dtypes

import concourse.bass as bass
import concourse.mybir as mybir
import concourse.tile as tile
from concourse import bacc
from concourse import bass_utils

BF16 = mybir.dt.bfloat16
F32 = mybir.dt.float32
AF = mybir.ActivationFunctionType

B, T, D = 4, 2048, 1024
H, DK = 16, 64
HALF = 512            # channels per core (8 heads)
KB = D // 128         # 8 contraction blocks for projections
TB = T // 128         # 16 t/k blocks of 128
QB = T // 512         # 4 query blocks of 512
NPAIR = 4             # head pairs per core (2 heads = 128 channels)
SCALE = float(DK) ** -0.5

N_CORES = 8

_PROG = None  # compiled program cache


def _build_program():
    nc = bacc.Bacc("TRN2", target_bir_lowering=False, debug=False)

    xt_d = nc.dram_tensor("xt", [KB, 128, T], BF16, kind="ExternalInput")
    wqt_d = nc.dram_tensor("wqt", [KB, 128, HALF], BF16, kind="ExternalInput")
    wkt_d = nc.dram_tensor("wkt", [KB, 128, HALF], BF16, kind="ExternalInput")
    wvt_d = nc.dram_tensor("wvt", [KB, 128, HALF], BF16, kind="ExternalInput")
    wot_d = nc.dram_tensor("wot", [4, 128, D], BF16, kind="ExternalInput")
    mask_d = nc.dram_tensor("mask", [128, 4, 1024], BF16, kind="ExternalInput")
    out_d = nc.dram_tensor("out", [TB, 128, D], F32, kind="ExternalOutput")

    with tile.TileContext(nc) as tc:
        with (
            tc.tile_pool(name="const", bufs=1) as const,
            tc.tile_pool(name="sb_pt", bufs=4) as sb_pt,
            tc.tile_pool(name="sb_otu", bufs=8) as sb_otu,
            tc.tile_pool(name="sb_lr", bufs=4) as sb_lr,
            tc.tile_pool(name="sb_rbr", bufs=4) as sb_rbr,
            tc.tile_pool(name="sb_sc", bufs=2) as sb_sc,
            tc.tile_pool(name="sb_out", bufs=2) as sb_out,
            tc.tile_pool(name="ps_st", bufs=2, space="PSUM") as ps_st,
            tc.tile_pool(name="ps_ot", bufs=2, space="PSUM") as ps_ot,
            tc.tile_pool(name="ps_acc", bufs=1, space="PSUM") as ps_acc,
        ):
            xt_sb = const.tile([128, KB, T], BF16, tag="xt")
            wqt_sb = const.tile([128, KB, HALF], BF16, tag="wqt")
            wkt_sb = const.tile([128, KB, HALF], BF16, tag="wkt")
            wvt_sb = const.tile([128, KB, HALF], BF16, tag="wvt")
            wot_sb = const.tile([128, 4, D], BF16, tag="wot")
            mask_sb = const.tile([128, 4, 1024], BF16, tag="mask")
            qt_sb = const.tile([128, NPAIR, T], BF16, tag="qt")
            kt_sb = const.tile([128, NPAIR, T], BF16, tag="kt")
            vaug_sb = const.tile([128, TB, 8 * 65], BF16, tag="vaug")
            otn_sb = const.tile([128, NPAIR, T], BF16, tag="otn")

            # fine-grained input DMAs round-robined over all five engine
            # queues (parallel HW-DGE queues), ordered so the first QT
            # accumulation chain (needs wqt + xt) can start as early as
            # possible
            engs = [nc.sync, nc.scalar, nc.gpsimd]
            _n = [0]

            def dma_in(dst, src):
                engs[_n[0] % len(engs)].dma_start(dst, src)
                _n[0] += 1

            nc.sync.dma_start(mask_sb[:], mask_d.ap())
            # HAM warm-up: keep TensorE busy during the input-DMA ramp so the
            # clock gate reaches 2.4 GHz before the real matmuls start
            warm = ps_acc.tile([128, 512], F32, tag="acc")
            for w in range(10):
                nc.tensor.matmul(
                    warm[:],
                    mask_sb[:, 0, 0:128],
                    mask_sb[:, 0, 0:512],
                    start=(w == 0),
                    stop=(w == 9),
                )
            for kb in range(KB):
                dma_in(wqt_sb[:, kb, :], wqt_d.ap()[kb])
                for nb in range(2):
                    tsl = slice(nb * 512, (nb + 1) * 512)
                    dma_in(xt_sb[:, kb, tsl], xt_d.ap()[kb][:, tsl])
            for kb in range(KB):
                dma_in(wkt_sb[:, kb, :], wkt_d.ap()[kb])
                for nb in range(2, 4):
                    tsl = slice(nb * 512, (nb + 1) * 512)
                    dma_in(xt_sb[:, kb, tsl], xt_d.ap()[kb][:, tsl])
            for kb in range(KB):
                dma_in(wvt_sb[:, kb, :], wvt_d.ap()[kb])
            for cb in range(4):
                dma_in(wot_sb[:, cb, :], wot_d.ap()[cb])
            for h in range(8):  # ones column per head in V_aug
                nc.vector.memset(vaug_sb[:, :, h * 65 + 64 : h * 65 + 65], 1.0)

            def emit_qk_proj(pair, nbps=(0, 1)):
                for dst_sb, w_sb in ((qt_sb, wqt_sb), (kt_sb, wkt_sb)):
                    for nbp in nbps:  # pairs of 512-t blocks
                        acc = ps_acc.tile([128, 1024], F32, tag="acc")
                        for kb in range(KB):
                            lhs = w_sb[:, kb, pair * 128 : (pair + 1) * 128]
                            nc.tensor.matmul(
                                acc[:, 0:512],
                                lhs,
                                xt_sb[:, kb, nbp * 1024 : nbp * 1024 + 512],
                                start=(kb == 0),
                                stop=(kb == KB - 1),
                            )
                            nc.tensor.matmul(
                                acc[:, 512:1024],
                                lhs,
                                xt_sb[:, kb, nbp * 1024 + 512 : (nbp + 1) * 1024],
                                start=(kb == 0),
                                stop=(kb == KB - 1),
                            )
                        nc.vector.tensor_copy(
                            dst_sb[:, pair, nbp * 1024 : (nbp + 1) * 1024], acc[:]
                        )

            def emit_v_proj(tbps):
                for tbp in tbps:  # pairs of 128-t blocks
                    acc = ps_acc.tile([128, 1024], F32, tag="acc")
                    for kb in range(KB):
                        nc.tensor.matmul(
                            acc[:, 0:512],
                            xt_sb[:, kb, (2 * tbp) * 128 : (2 * tbp + 1) * 128],
                            wvt_sb[:, kb, :],
                            start=(kb == 0),
                            stop=(kb == KB - 1),
                        )
                        nc.tensor.matmul(
                            acc[:, 512:1024],
                            xt_sb[:, kb, (2 * tbp + 1) * 128 : (2 * tbp + 2) * 128],
                            wvt_sb[:, kb, :],
                            start=(kb == 0),
                            stop=(kb == KB - 1),
                        )
                    nc.vector.tensor_copy(
                        vaug_sb[:, 2 * tbp : 2 * tbp + 2, :].rearrange(
                            "p a (h c) -> p a h c", c=65
                        )[:, :, :, 0:64],
                        acc[:].rearrange("p (a h c) -> p a h c", a=2, c=64),
                    )

            def emit_attention(pair, qb):
                h0 = 2 * pair
                if True:
                    jmax = 4 * qb + 3
                    qsl = slice(qb * 512, (qb + 1) * 512)
                    ot0 = ps_ot.tile([65, 512], F32, tag="ot")
                    ot1 = ps_ot.tile([65, 512], F32, tag="ot")
                    for j in range(jmax + 1):
                        jsl = slice(j * 128, (j + 1) * 128)
                        d = j - 4 * qb
                        # columns q < 128*d of this tile are fully masked:
                        # skip the ST matmul / exp work there entirely
                        lo = 128 * d if d >= 1 else 0
                        vq = slice(qb * 512 + lo, (qb + 1) * 512)
                        st = ps_st.tile([128, 1024], F32, tag="st")
                        st3 = st[:].rearrange("p (h q) -> p h q", h=2)
                        nc.tensor.matmul(
                            st3[:, 0, lo:512], kt_sb[0:64, pair, jsl], qt_sb[0:64, pair, vq]
                        )
                        nc.tensor.matmul(
                            st3[:, 1, lo:512],
                            kt_sb[64:128, pair, jsl],
                            qt_sb[64:128, pair, vq],
                        )
                        pt = sb_pt.tile([128, 1024], BF16, tag="pt")
                        pt3 = pt[:].rearrange("p (h q) -> p h q", h=2)
                        nc.scalar.activation(
                            pt3[:, :, lo:512], st3[:, :, lo:512], AF.Exp, scale=SCALE
                        )
                        if d >= 0:
                            # only the 128-wide staircase band [lo, lo+128)
                            # is partially masked; columns below lo are
                            # skipped by the partial-N PV matmuls entirely
                            nc.vector.tensor_mul(
                                pt3[:, :, lo : lo + 128],
                                pt3[:, :, lo : lo + 128],
                                mask_sb[:, 0, :].rearrange(
                                    "p (h q) -> p h q", h=2
                                )[:, :, 0:128],
                            )
                        nc.tensor.matmul(
                            ot0[:, lo:512],
                            vaug_sb[:, j, h0 * 65 : (h0 + 1) * 65],
                            pt3[:, 0, lo:512],
                            start=(j == 0),
                            stop=(j == jmax),
                        )
                        nc.tensor.matmul(
                            ot1[:, lo:512],
                            vaug_sb[:, j, (h0 + 1) * 65 : (h0 + 2) * 65],
                            pt3[:, 1, lo:512],
                            start=(j == 0),
                            stop=(j == jmax),
                        )
                    for hh, ot in ((0, ot0), (1, ot1)):
                        otu = sb_otu.tile([64, 512], BF16, tag="otu")
                        nc.vector.tensor_copy(otu[:], ot[0:64, :])
                        lrow = sb_lr.tile([1, 512], F32, tag="lrow")
                        nc.vector.tensor_copy(lrow[:], ot[64:65, :])
                        rec = sb_lr.tile([1, 512], F32, tag="rec")
                        nc.vector.reciprocal_approx_fast(rec[:], lrow[:])
                        rbr = sb_rbr.tile([64, 512], F32, tag="rbr")
                        nc.gpsimd.partition_broadcast(rbr[:], rec[0:1, :])
                        if hh == 0:
                            nc.vector.tensor_mul(
                                otn_sb[0:64, pair, qsl], otu[:], rbr[:]
                            )
                        else:
                            sc = sb_sc.tile([64, 512], BF16, tag="sc")
                            nc.vector.tensor_mul(sc[:], otu[:], rbr[:])
                            nc.sync.dma_start(otn_sb[64:128, pair, qsl], sc[:])

            def emit_out_proj(tb):
                tsl = slice(tb * 128, (tb + 1) * 128)
                acc = ps_acc.tile([128, 1024], F32, tag="acc")
                for cb in range(4):
                    lhs = otn_sb[:, cb, tsl]
                    nc.tensor.matmul(
                        acc[:, 0:512],
                        lhs,
                        wot_sb[:, cb, 0:512],
                        start=(cb == 0),
                        stop=(cb == 3),
                    )
                    nc.tensor.matmul(
                        acc[:, 512:1024],
                        lhs,
                        wot_sb[:, cb, 512:1024],
                        start=(cb == 0),
                        stop=(cb == 3),
                    )
                outc = sb_out.tile([128, D], F32, tag="outc")
                nc.scalar.copy(outc[:], acc[:])
                [nc.sync, nc.gpsimd][tb % 2].dma_start(out_d.ap()[tb], outc[:])

            # Emission order: A(p) ahead of B(p) so the next pair's
            # projection matmuls fill TensorE gaps while ACT paces B; the
            # phase-C tiles for query block qb are emitted right after
            # B(p3, qb) (all their otn inputs are complete by then) so they
            # fill TensorE gaps during B(p3) instead of bunching at the end.
            for pair in range(NPAIR):
                if pair == 0:
                    # order p0's projections so B(p0, qb0/qb1) dependencies
                    # (t-cols 0:1024 of QT/KT, V t-blocks 0:8) finish first
                    emit_qk_proj(0, nbps=(0,))
                    emit_v_proj(range(4))
                    emit_qk_proj(0, nbps=(1,))
                    emit_v_proj(range(4, 8))
                else:
                    emit_qk_proj(pair)
                for qb in range(QB):
                    emit_attention(pair, qb)
                    if pair == NPAIR - 1:
                        for tb in range(4 * qb, 4 * qb + 4):
                            emit_out_proj(tb)

    nc.compile()
    return nc


def _prep_core_inputs(X, W_q, W_k, W_v, W_o, mask_host, c):
    b, half = c // 2, c % 2
    ch = slice(half * HALF, (half + 1) * HALF)
    bf = ml_dtypes.bfloat16
    xt = np.ascontiguousarray(X[b].T).reshape(KB, 128, T).astype(bf)
    wqt = np.ascontiguousarray(W_q[ch, :].T).reshape(KB, 128, HALF).astype(bf)
    wkt = np.ascontiguousarray(W_k[ch, :].T).reshape(KB, 128, HALF).astype(bf)
    wvt = np.ascontiguousarray(W_v[ch, :].T).reshape(KB, 128, HALF).astype(bf)
    wot = np.ascontiguousarray(W_o[:, ch].T).reshape(4, 128, D).astype(bf)
    return {
        "xt": xt, "wqt": wqt, "wkt": wkt, "wvt": wvt, "wot": wot,
        "mask": mask_host,
    }


def _make_mask():
    kp = np.arange(128)[:, None]
    qf = np.arange(512)[None, :]
    m = np.zeros((128, 4, 1024), np.float32)
    for d in range(4):
        keep = (qf >= kp + d * 128).astype(np.float32)
        m[:, d, 0:512] = keep
        m[:, d, 512:1024] = keep
    return m.astype(ml_dtypes.bfloat16)


def kernel(X, W_q, W_k, W_v, W_o):
    global _PROG
    X = np.asarray(X, dtype=np.float32)
    W_q = np.asarray(W_q, dtype=np.float32)
    W_k = np.asarray(W_k, dtype=np.float32)
    W_v = np.asarray(W_v, dtype=np.float32)
    W_o = np.asarray(W_o, dtype=np.float32)

    if _PROG is None:
        _PROG = _build_program()
    nc = _PROG

    mask_host = _make_mask()
    in_maps = [
        _prep_core_inputs(X, W_q, W_k, W_v, W_o, mask_host, c)
        for c in range(N_CORES)
    ]
    res = bass_utils.run_bass_kernel_spmd(nc, in_maps, core_ids=list(range(N_CORES)))

    out = np.empty((B, T, D), np.float32)
    for b in range(B):
        p0 = res.results[2 * b]["out"].reshape(T, D)
        p1 = res.results[2 * b + 1]["out"].reshape(T, D)
        out[b] = p0 + p1
    return out
